# revision 13
# baseline (speedup 1.0000x reference)
"""Trainium2 Bass kernel for nn_BartPooler_53815940219079 (segment_reduce).

Computes, for each of B*T segments of a [B, S, H] hidden-state tensor:
  feat = concat([segment_max, segment_mean])  -> tanh(feat @ W.T + b)

Strategy (8 NeuronCores, SPMD — one program, per-core data):
  * Host compacts each segment's used tokens into a per-core token stream
    shipped PRE-TRANSPOSED as [128 h-partitions, 8 h-blocks, tokens] fp16.
    Segments are grouped into BANDS of similar length; every slot in a band
    is padded to the band's uniform length Lb (dup of the segment's first
    token), so each band is a [128, 8*ns, Lb] tile with all slots aligned.
  * Per band, segment max AND sum are log2 halving trees of tensor_tensor
    ops (fp16 hits the DVE 2x perf mode; one instruction per level covers
    every slot in the band) + one final tensor_reduce.  ~25% of bands run
    their trees on GpSimd to unload the VectorE.  The dup-padding bias in
    the sum is cancelled with mean = alpha*sum + beta*first_token, where
    first_token is read from the band's last (padded) column.
  * Final [2H]x[2H,D] GEMM: 16 k-blocks packed 4-up into PE column
    quadrants (M=32 each), accumulated 4 steps deep, one fold matmul per
    512-col half, pipelined behind the 4 W DMA chunks; then bias + tanh.
"""

import numpy as np

import concourse.bacc as bacc
import concourse.mybir as mybir
import concourse.tile as tile
from concourse.bass_utils import run_bass_kernel_spmd

NCORES = 8
B, S, H, T = 16, 4096, 1024, 16
D_OUT = 1024
HB = H // 128  # h-blocks per hidden vector

F32 = mybir.dt.float32
F16 = mybir.dt.float16

SPREAD = 0.08      # band break threshold on slot-length spread
GP_SHARE = 0.5     # fraction of elements whose SUM tree runs on GpSimd


def _levels(lb):
    lv = 0
    while lv < 4 and (lb >> (lv + 1)) >= 4:
        lv += 1
    return lv


def _build_schedule(parts, turns):
    """Host-side: segment list -> per-core banded layout (uniform shapes)."""
    Bn, Tn = parts.shape
    segs = []  # (global_row, example, start_token, count)
    for b in range(Bn):
        cum = 0
        for j in range(Tn):
            c = int(parts[b, j])
            if j < int(turns[b]):
                segs.append((b * Tn + j, b, 1 + cum, c))
            cum += c

    # Deal segments to cores by size rank: slot j holds the segments of
    # ranks [NC*j, NC*j+NC), one per core, so per-slot lengths stay close.
    order = sorted(range(len(segs)), key=lambda i: -segs[i][3])
    core_slots = [[] for _ in range(NCORES)]
    for rank, i in enumerate(order):
        core_slots[rank % NCORES].append(segs[i])
    ncap = max(len(s) for s in core_slots)
    assert ncap <= 32, ncap

    L = [max(core_slots[c][j][3] for c in range(NCORES)
             if j < len(core_slots[c])) for j in range(ncap)]

    # Bands: consecutive slots whose lengths are within SPREAD of the
    # band leader, padded to uniform Lb (multiple of 2^levels).
    bands = []  # (j0, j1, o0, Lb, lv)
    off = 0
    j0 = 0
    for j in range(1, ncap + 1):
        if j == ncap or (L[j0] - L[j]) > SPREAD * L[j0] + 2:
            lead = max(8, L[j0])
            lv = _levels(lead)
            m = 1 << lv
            lb = -(-lead // m) * m
            bands.append((j0, j, off, lb, lv))
            off += (j - j0) * lb
            j0 = j
    ntok = off

    # Per-band SUM-tree assignment to GpSimd (Pool rejects max, so the max
    # trees always run on VectorE).
    total_e = sum((j1 - j0) * lb for (j0, j1, _, lb, _) in bands)
    gp_bands = set()
    acc = 0
    for k, (j0, j1, _, lb, _) in enumerate(bands):
        if acc / total_e < GP_SHARE:
            gp_bands.add(k)
            acc += (j1 - j0) * lb
    A = np.zeros(ncap, dtype=np.int64)  # slot -> token start
    LB = np.zeros(ncap, dtype=np.int64)
    for (j0, j1, o0, lb, lv) in bands:
        for j in range(j0, j1):
            A[j] = o0 + (j - j0) * lb
            LB[j] = lb

    tok_idx = np.zeros((NCORES, ntok), dtype=np.int64)
    alpha = np.zeros((NCORES, ncap), dtype=np.float32)
    beta = np.zeros((NCORES, ncap), dtype=np.float32)
    out_map = np.full((NCORES, ncap), -1, dtype=np.int64)
    for c in range(NCORES):
        for j, (grow, b, s0, cnt) in enumerate(core_slots[c]):
            out_map[c, j] = grow
            a, lb = int(A[j]), int(LB[j])
            base = b * S + s0
            tok_idx[c, a:a + cnt] = np.arange(base, base + cnt)
            tok_idx[c, a + cnt:a + lb] = base  # dup first token
            alpha[c, j] = 1.0 / cnt
            beta[c, j] = -(lb - cnt) / cnt
    return {
        "ncap": ncap, "ntok": ntok, "bands": bands, "A": A, "LB": LB,
        "gp_bands": gp_bands,
        "tok_idx": tok_idx, "alpha": alpha, "beta": beta,
        "out_map": out_map, "nrows": Bn * Tn,
    }


def _build_program(sched):
    """Emit the SPMD Bass program (identical for all cores)."""
    ncap, ntok, bands = sched["ncap"], sched["ntok"], sched["bands"]
    gp_bands = sched["gp_bands"]

    nc = bacc.Bacc("TRN2", target_bir_lowering=False, debug=False,
                   num_devices=NCORES)
    hid = nc.dram_tensor("hid", [128, HB * ntok], F16, kind="ExternalInput")
    wt = nc.dram_tensor("wt", [128, 4, 4, D_OUT], F16, kind="ExternalInput")
    alpha = nc.dram_tensor("alpha", [128, HB, ncap], F32, kind="ExternalInput")
    beta = nc.dram_tensor("beta", [128, ncap], F32, kind="ExternalInput")
    brep = nc.dram_tensor("brep", [ncap, D_OUT], F32, kind="ExternalInput")
    fold = nc.dram_tensor("fold", [128, ncap], F16, kind="ExternalInput")
    out = nc.dram_tensor("out", [ncap, D_OUT], F32, kind="ExternalOutput")

    with tile.TileContext(nc) as tc:
        with (
            tc.tile_pool(name="main", bufs=1) as mp,
            tc.tile_pool(name="psum", bufs=1, space="PSUM") as pp,
        ):
            # Aux on the scalar queue (tiny, needed by merges/GEMM epilogue).
            alpha_sb = mp.tile([128, HB, ncap], F32)
            nc.scalar.dma_start(out=alpha_sb[:], in_=alpha[:])
            beta_sb = mp.tile([128, ncap], F32)
            nc.scalar.dma_start(out=beta_sb[:], in_=beta[:])
            brep_sb = mp.tile([ncap, D_OUT], F32)
            nc.scalar.dma_start(out=brep_sb[:], in_=brep[:])
            fold_sb = mp.tile([128, ncap], F16)
            nc.scalar.dma_start(out=fold_sb[:], in_=fold[:])

            # Stream bands on the in-order sync queue; W rides behind them.
            band_tiles = []
            for k, (j0, j1, o0, lb, lv) in enumerate(bands):
                ns = j1 - j0
                bt = mp.tile([128, HB, ns * lb], F16, name=f"band{k}",
                             tag=f"band{k}")
                nc.sync.dma_start(
                    out=bt[:].rearrange("p b e -> p (b e)"),
                    in_=hid[:, HB * o0:HB * (o0 + ns * lb)],
                )
                band_tiles.append(bt)
            wt_sb = mp.tile([128, 4, 4, D_OUT], F16)
            wt_dmas = []
            for step in range(4):
                wt_dmas.append(nc.sync.dma_start(
                    out=wt_sb[:, step], in_=wt[:, step]))

            maxT = mp.tile([128, HB, ncap], F16)
            sumT = mp.tile([128, HB, ncap], F32)
            firsts = mp.tile([128, HB, ncap], F32)
            mtmp = mp.tile([128, HB, ncap], F32)
            meansT = mp.tile([128, HB, ncap], F16)

            for k, (j0, j1, o0, lb, lv) in enumerate(bands):
                ns = j1 - j0
                q = HB * ns
                # Band data is token-major: [128, lb, (hb, slot)] flattened,
                # so every halving is a contiguous 2D op on either engine.
                flat = band_tiles[k][:].rearrange("p b e -> p (b e)")
                sum_eng = nc.gpsimd if k in gp_bands else nc.vector
                sc = mp.tile([128, q * (lb // 2)], F16, name=f"sc{k}",
                             tag=f"sc{k}")

                def qt(buf, width):
                    # [128, width*q] token-major -> [128, q, width] view
                    return buf[:, :width * q].rearrange(
                        "p (t q) -> p q t", q=q)

                # beta * first_token: the band's last token row is the dup of
                # the first token whenever padding exists (beta=0 otherwise),
                # and the trees below never write it.
                lastrow = flat[:, (lb - 1) * q:lb * q].rearrange(
                    "p (b s) -> p b s", b=HB)
                for j in range(j0, j1):
                    nc.scalar.activation(firsts[:, :, j:j + 1],
                                         lastrow[:, :, j - j0:j - j0 + 1],
                                         mybir.ActivationFunctionType.Copy,
                                         scale=beta_sb[:, j:j + 1])

                # max tree (VectorE; Pool rejects max): L1 into scratch so
                # the sum tree's in-place writes on the band tile can't stall
                # it, then in-place halving on the scratch.
                m = lb // 2
                nc.vector.tensor_tensor(out=sc[:], in0=flat[:, :m * q],
                                        in1=flat[:, m * q:lb * q],
                                        op=mybir.AluOpType.max)
                for _ in range(1, lv):
                    nc.vector.tensor_tensor(out=sc[:, :m // 2 * q],
                                            in0=sc[:, :m // 2 * q],
                                            in1=sc[:, m // 2 * q:m * q],
                                            op=mybir.AluOpType.max)
                    m //= 2
                nc.vector.reduce_max(out=maxT[:, :, j0:j1], in_=qt(sc, m),
                                     axis=mybir.AxisListType.X)
                # sum tree (GpSimd for ~half the elements): in place on the
                # band tile, whose lower half is only read by the max L1.
                w = lb // 2
                sum_eng.tensor_tensor(out=flat[:, :w * q],
                                      in0=flat[:, :w * q],
                                      in1=flat[:, w * q:lb * q],
                                      op=mybir.AluOpType.add)
                for _ in range(1, lv):
                    sum_eng.tensor_tensor(out=flat[:, :w // 2 * q],
                                          in0=flat[:, :w // 2 * q],
                                          in1=flat[:, w // 2 * q:w * q],
                                          op=mybir.AluOpType.add)
                    w //= 2
                nc.vector.reduce_sum(out=sumT[:, :, j0:j1], in_=qt(flat, w),
                                     axis=mybir.AxisListType.X)
                sl = slice(j0, j1)
                nc.vector.tensor_tensor(out=mtmp[:, :, sl],
                                        in0=sumT[:, :, sl],
                                        in1=alpha_sb[:, :, sl],
                                        op=mybir.AluOpType.mult)
                nc.vector.tensor_tensor(out=meansT[:, :, sl],
                                        in0=mtmp[:, :, sl],
                                        in1=firsts[:, :, sl],
                                        op=mybir.AluOpType.add)

            # GEMM: out[slot, n] = sum_k featT[k, slot] * wt[k, n], feat =
            # [max | mean].  kb = 4*cg + step; the 4 column groups stream
            # concurrently, each accumulating its 4 k-blocks into its own
            # partition quadrant; chunk `step` of W unlocks step-i matmuls.
            osb = mp.tile([ncap, D_OUT], F32)
            for nh in range(2):
                nsl = slice(nh * 512, (nh + 1) * 512)
                gem_ps = pp.tile([128, 512], F32, name=f"gem{nh}")
                for step in range(4):
                    for cg in range(4):
                        kb = 4 * cg + step
                        lhsT = (maxT[:, kb, :] if kb < HB
                                else meansT[:, kb - HB, :])
                        nc.tensor.matmul(
                            gem_ps[32 * cg:32 * cg + ncap, :],
                            lhsT=lhsT,
                            rhs=wt_sb[:, step, cg, nsl],
                            start=(step == 0),
                            stop=(step == 3),
                            tile_position=(0, 32 * cg),
                        )
                gem_sb = mp.tile([128, 512], F16, name=f"gsb{nh}")
                nc.scalar.copy(out=gem_sb[:], in_=gem_ps[:])
                fold_ps = pp.tile([ncap, 512], F32, name=f"fps{nh}")
                nc.tensor.matmul(fold_ps[:], lhsT=fold_sb[:, :ncap],
                                 rhs=gem_sb[:], start=True, stop=True)
                nc.vector.tensor_add(out=osb[:, nsl], in0=fold_ps[:],
                                     in1=brep_sb[:, nsl])
            nc.scalar.activation(osb[:], osb[:],
                                 mybir.ActivationFunctionType.Tanh)
            nc.sync.dma_start(out=out[:], in_=osb[:])

    nc.compile()
    return nc


def _build_in_maps(sched, hidden_states, W, b):
    ncap, ntok, bands = sched["ncap"], sched["ntok"], sched["bands"]
    flat16 = np.ascontiguousarray(
        np.asarray(hidden_states).reshape(B * S, H)).astype(np.float16)
    # W.T rows permuted so chunk `step` holds kb = 4*cg + step, cg=0..3.
    WT = np.asarray(W, dtype=np.float32).T.reshape(2 * HB, 128, D_OUT)
    wt_np = np.zeros((128, 4, 4, D_OUT), np.float32)
    for step in range(4):
        for cg in range(4):
            wt_np[:, step, cg, :] = WT[4 * cg + step]
    wt_np = np.ascontiguousarray(wt_np).astype(np.float16)
    brep_np = np.ascontiguousarray(
        np.broadcast_to(np.asarray(b, dtype=np.float32), (ncap, D_OUT)))
    fold_np = np.zeros((128, ncap), np.float16)
    for cg in range(4):
        for j in range(ncap):
            fold_np[32 * cg + j, j] = 1.0

    in_maps = []
    for c in range(NCORES):
        tok = flat16[sched["tok_idx"][c]]                      # [ntok, H]
        arr = np.ascontiguousarray(
            tok.T.reshape(HB, 128, ntok).transpose(1, 0, 2))   # [128, HB, ntok]
        pieces = []
        for k, (j0, j1, o0, lb, lv) in enumerate(bands):
            ns = j1 - j0
            ba = arr[:, :, o0:o0 + ns * lb]                    # [128, HB, ns*lb]
            # token-major: [128, lb, HB*ns]
            ba = ba.reshape(128, HB, ns, lb).transpose(0, 3, 1, 2)
            pieces.append(np.ascontiguousarray(ba).reshape(128, -1))
        hid_np = np.concatenate(pieces, axis=1)
        alpha_np = np.ascontiguousarray(np.broadcast_to(
            sched["alpha"][c][None, None, :], (128, HB, ncap)))
        beta_np = np.ascontiguousarray(np.broadcast_to(
            sched["beta"][c][None, :], (128, ncap)))
        in_maps.append({
            "hid": np.ascontiguousarray(hid_np),
            "wt": wt_np,
            "alpha": alpha_np,
            "beta": beta_np,
            "brep": brep_np,
            "fold": fold_np,
        })
    return in_maps


def kernel(hidden_states, W, b, turns, parts):
    parts = np.asarray(parts)
    turns = np.asarray(turns)

    sched = _build_schedule(parts, turns)
    nc = _build_program(sched)
    in_maps = _build_in_maps(sched, hidden_states, W, b)

    res = run_bass_kernel_spmd(nc, in_maps, list(range(NCORES)))

    full = np.zeros((sched["nrows"], D_OUT), dtype=np.float32)
    for c in range(NCORES):
        oc = res.results[c]["out"]
        for j in range(sched["ncap"]):
            g = sched["out_map"][c, j]
            if g >= 0:
                full[g] = oc[j]
    return full


# revision 15
# speedup vs baseline: 1.3146x; 1.3146x over previous
"""Trainium2 Bass kernel for nn_BartPooler_53815940219079 (segment_reduce).

Computes, for each of B*T segments of a [B, S, H] hidden-state tensor:
  feat = concat([segment_max, segment_mean])  -> tanh(feat @ W.T + b)

Strategy (8 NeuronCores, SPMD — one program, per-core data):
  * Host compacts each segment's used tokens into a per-core token stream
    shipped PRE-TRANSPOSED as [128 h-partitions, 8 h-blocks, tokens] fp16.
    Segments are grouped into BANDS of similar length; every slot in a band
    is padded to the band's uniform length Lb (dup of the segment's first
    token), so each band is a [128, 8*ns, Lb] tile with all slots aligned.
  * Per band, segment max AND sum are log2 halving trees of tensor_tensor
    ops (fp16 hits the DVE 2x perf mode; one instruction per level covers
    every slot in the band) + one final tensor_reduce.  ~25% of bands run
    their trees on GpSimd to unload the VectorE.  The dup-padding bias in
    the sum is cancelled with mean = alpha*sum + beta*first_token, where
    first_token is read from the band's last (padded) column.
  * Final [2H]x[2H,D] GEMM: 16 k-blocks packed 4-up into PE column
    quadrants (M=32 each), accumulated 4 steps deep, one fold matmul per
    512-col half, pipelined behind the 4 W DMA chunks; then bias + tanh.
"""

import numpy as np

import concourse.bacc as bacc
import concourse.mybir as mybir
import concourse.tile as tile
from concourse.bass_utils import run_bass_kernel_spmd

NCORES = 8
B, S, H, T = 16, 4096, 1024, 16
D_OUT = 1024
HB = H // 128  # h-blocks per hidden vector

F32 = mybir.dt.float32
F16 = mybir.dt.float16

SPREAD = 0.15      # band break threshold on slot-length spread
GP_SHARE = 0.0     # GpSimd concurrency starves DVE ~8x; keep it idle


def _levels(lb):
    lv = 0
    while lv < 4 and (lb >> (lv + 1)) >= 4:
        lv += 1
    return lv


def _build_schedule(parts, turns):
    """Host-side: segment list -> per-core banded layout (uniform shapes)."""
    Bn, Tn = parts.shape
    segs = []  # (global_row, example, start_token, count)
    for b in range(Bn):
        cum = 0
        for j in range(Tn):
            c = int(parts[b, j])
            if j < int(turns[b]):
                segs.append((b * Tn + j, b, 1 + cum, c))
            cum += c

    # Deal segments to cores by size rank: slot j holds the segments of
    # ranks [NC*j, NC*j+NC), one per core, so per-slot lengths stay close.
    order = sorted(range(len(segs)), key=lambda i: -segs[i][3])
    core_slots = [[] for _ in range(NCORES)]
    for rank, i in enumerate(order):
        core_slots[rank % NCORES].append(segs[i])
    ncap = max(len(s) for s in core_slots)
    assert ncap <= 32, ncap

    L = [max(core_slots[c][j][3] for c in range(NCORES)
             if j < len(core_slots[c])) for j in range(ncap)]

    # Bands: consecutive slots whose lengths are within SPREAD of the
    # band leader, padded to uniform Lb (multiple of 2^levels).
    bands = []  # (j0, j1, o0, Lb, lv)
    off = 0
    j0 = 0
    for j in range(1, ncap + 1):
        if j == ncap or (L[j0] - L[j]) > SPREAD * L[j0] + 2:
            lead = max(8, L[j0])
            lv = _levels(lead)
            m = 1 << lv
            lb = -(-lead // m) * m
            bands.append((j0, j, off, lb, lv))
            off += (j - j0) * lb
            j0 = j
    ntok = off

    # Per-band SUM-tree assignment to GpSimd (Pool rejects max, so the max
    # trees always run on VectorE).
    total_e = sum((j1 - j0) * lb for (j0, j1, _, lb, _) in bands)
    gp_bands = set()
    acc = 0
    for k, (j0, j1, _, lb, _) in enumerate(bands):
        if acc / total_e < GP_SHARE:
            gp_bands.add(k)
            acc += (j1 - j0) * lb
    A = np.zeros(ncap, dtype=np.int64)  # slot -> token start
    LB = np.zeros(ncap, dtype=np.int64)
    for (j0, j1, o0, lb, lv) in bands:
        for j in range(j0, j1):
            A[j] = o0 + (j - j0) * lb
            LB[j] = lb

    tok_idx = np.zeros((NCORES, ntok), dtype=np.int64)
    alpha = np.zeros((NCORES, ncap), dtype=np.float32)
    beta = np.zeros((NCORES, ncap), dtype=np.float32)
    out_map = np.full((NCORES, ncap), -1, dtype=np.int64)
    for c in range(NCORES):
        for j, (grow, b, s0, cnt) in enumerate(core_slots[c]):
            out_map[c, j] = grow
            a, lb = int(A[j]), int(LB[j])
            base = b * S + s0
            tok_idx[c, a:a + cnt] = np.arange(base, base + cnt)
            tok_idx[c, a + cnt:a + lb] = base  # dup first token
            alpha[c, j] = 1.0 / cnt
            beta[c, j] = -(lb - cnt) / cnt
    return {
        "ncap": ncap, "ntok": ntok, "bands": bands, "A": A, "LB": LB,
        "gp_bands": gp_bands,
        "tok_idx": tok_idx, "alpha": alpha, "beta": beta,
        "out_map": out_map, "nrows": Bn * Tn,
    }


def _build_program(sched):
    """Emit the SPMD Bass program (identical for all cores)."""
    ncap, ntok, bands = sched["ncap"], sched["ntok"], sched["bands"]
    gp_bands = sched["gp_bands"]

    nc = bacc.Bacc("TRN2", target_bir_lowering=False, debug=False,
                   num_devices=NCORES)
    hid = nc.dram_tensor("hid", [128, HB * ntok], F16, kind="ExternalInput")
    wt = nc.dram_tensor("wt", [128, 4, 4, D_OUT], F16, kind="ExternalInput")
    alpha = nc.dram_tensor("alpha", [128, HB, ncap], F32, kind="ExternalInput")
    beta = nc.dram_tensor("beta", [128, ncap], F32, kind="ExternalInput")
    brep = nc.dram_tensor("brep", [ncap, D_OUT], F32, kind="ExternalInput")
    fold = nc.dram_tensor("fold", [128, ncap], F16, kind="ExternalInput")
    out = nc.dram_tensor("out", [ncap, D_OUT], F32, kind="ExternalOutput")

    with tile.TileContext(nc) as tc:
        with (
            tc.tile_pool(name="main", bufs=1) as mp,
            tc.tile_pool(name="psum", bufs=1, space="PSUM") as pp,
        ):
            # Aux on the scalar queue (tiny, needed by merges/GEMM epilogue).
            alpha_sb = mp.tile([128, HB, ncap], F32)
            nc.scalar.dma_start(out=alpha_sb[:], in_=alpha[:])
            beta_sb = mp.tile([128, ncap], F32)
            nc.scalar.dma_start(out=beta_sb[:], in_=beta[:])
            brep_sb = mp.tile([ncap, D_OUT], F32)
            nc.scalar.dma_start(out=brep_sb[:], in_=brep[:])
            fold_sb = mp.tile([128, ncap], F16)
            nc.scalar.dma_start(out=fold_sb[:], in_=fold[:])

            # Stream bands on the in-order sync queue; W rides behind them.
            band_tiles = []
            for k, (j0, j1, o0, lb, lv) in enumerate(bands):
                ns = j1 - j0
                bt = mp.tile([128, HB, ns * lb], F16, name=f"band{k}",
                             tag=f"band{k}")
                nc.sync.dma_start(
                    out=bt[:].rearrange("p b e -> p (b e)"),
                    in_=hid[:, HB * o0:HB * (o0 + ns * lb)],
                )
                band_tiles.append(bt)
            wt_sb = mp.tile([128, 4, 4, D_OUT], F16)
            wt_dmas = []
            for step in range(4):
                wt_dmas.append(nc.sync.dma_start(
                    out=wt_sb[:, step], in_=wt[:, step]))

            maxT = mp.tile([128, HB, ncap], F16)
            sumT = mp.tile([128, HB, ncap], F32)
            firsts = mp.tile([128, HB, ncap], F32)
            mtmp = mp.tile([128, HB, ncap], F32)
            meansT = mp.tile([128, HB, ncap], F16)

            for k, (j0, j1, o0, lb, lv) in enumerate(bands):
                ns = j1 - j0
                q = HB * ns
                # Band data is token-major: [128, lb, (hb, slot)] flattened,
                # so every halving is a contiguous 2D op on either engine.
                flat = band_tiles[k][:].rearrange("p b e -> p (b e)")
                sum_eng = nc.gpsimd if k in gp_bands else nc.vector
                sc = mp.tile([128, q * (lb // 2)], F16, name=f"sc{k}",
                             tag=f"sc{k}")

                def qt(buf, width):
                    # [128, width*q] token-major -> [128, q, width] view
                    return buf[:, :width * q].rearrange(
                        "p (t q) -> p q t", q=q)

                # beta * first_token: the band's last token row is the dup of
                # the first token whenever padding exists (beta=0 otherwise),
                # and the trees below never write it.
                lastrow = flat[:, (lb - 1) * q:lb * q].rearrange(
                    "p (b s) -> p b s", b=HB)
                for j in range(j0, j1):
                    nc.scalar.activation(firsts[:, :, j:j + 1],
                                         lastrow[:, :, j - j0:j - j0 + 1],
                                         mybir.ActivationFunctionType.Copy,
                                         scale=beta_sb[:, j:j + 1])

                # max tree (VectorE; Pool rejects max): L1 into scratch so
                # the sum tree's in-place writes on the band tile can't stall
                # it, then in-place halving on the scratch.
                m = lb // 2
                nc.vector.tensor_tensor(out=sc[:], in0=flat[:, :m * q],
                                        in1=flat[:, m * q:lb * q],
                                        op=mybir.AluOpType.max)
                for _ in range(1, lv):
                    nc.vector.tensor_tensor(out=sc[:, :m // 2 * q],
                                            in0=sc[:, :m // 2 * q],
                                            in1=sc[:, m // 2 * q:m * q],
                                            op=mybir.AluOpType.max)
                    m //= 2
                nc.vector.reduce_max(out=maxT[:, :, j0:j1], in_=qt(sc, m),
                                     axis=mybir.AxisListType.X)
                # sum tree (GpSimd for ~half the elements): in place on the
                # band tile, whose lower half is only read by the max L1.
                w = lb // 2
                sum_eng.tensor_tensor(out=flat[:, :w * q],
                                      in0=flat[:, :w * q],
                                      in1=flat[:, w * q:lb * q],
                                      op=mybir.AluOpType.add)
                for _ in range(1, lv):
                    sum_eng.tensor_tensor(out=flat[:, :w // 2 * q],
                                          in0=flat[:, :w // 2 * q],
                                          in1=flat[:, w // 2 * q:w * q],
                                          op=mybir.AluOpType.add)
                    w //= 2
                nc.vector.reduce_sum(out=sumT[:, :, j0:j1], in_=qt(flat, w),
                                     axis=mybir.AxisListType.X)
            # mean = alpha*sum + beta*first, merged once over all slots
            nc.vector.tensor_tensor(out=mtmp[:], in0=sumT[:],
                                    in1=alpha_sb[:],
                                    op=mybir.AluOpType.mult)
            nc.vector.tensor_tensor(out=meansT[:], in0=mtmp[:],
                                    in1=firsts[:],
                                    op=mybir.AluOpType.add)

            # GEMM: out[slot, n] = sum_k featT[k, slot] * wt[k, n], feat =
            # [max | mean].  kb = 4*cg + step; the 4 column groups stream
            # concurrently, each accumulating its 4 k-blocks into its own
            # partition quadrant; chunk `step` of W unlocks step-i matmuls.
            osb = mp.tile([ncap, D_OUT], F32)
            for nh in range(2):
                nsl = slice(nh * 512, (nh + 1) * 512)
                gem_ps = pp.tile([128, 512], F32, name=f"gem{nh}")
                for step in range(4):
                    for cg in range(4):
                        kb = 4 * cg + step
                        lhsT = (maxT[:, kb, :] if kb < HB
                                else meansT[:, kb - HB, :])
                        nc.tensor.matmul(
                            gem_ps[32 * cg:32 * cg + ncap, :],
                            lhsT=lhsT,
                            rhs=wt_sb[:, step, cg, nsl],
                            start=(step == 0),
                            stop=(step == 3),
                            tile_position=(0, 32 * cg),
                        )
                gem_sb = mp.tile([128, 512], F16, name=f"gsb{nh}")
                nc.scalar.copy(out=gem_sb[:], in_=gem_ps[:])
                fold_ps = pp.tile([ncap, 512], F32, name=f"fps{nh}")
                nc.tensor.matmul(fold_ps[:], lhsT=fold_sb[:, :ncap],
                                 rhs=gem_sb[:], start=True, stop=True)
                nc.vector.tensor_add(out=osb[:, nsl], in0=fold_ps[:],
                                     in1=brep_sb[:, nsl])
            nc.scalar.activation(osb[:], osb[:],
                                 mybir.ActivationFunctionType.Tanh)
            nc.sync.dma_start(out=out[:], in_=osb[:])

    nc.compile()
    return nc


def _build_in_maps(sched, hidden_states, W, b):
    ncap, ntok, bands = sched["ncap"], sched["ntok"], sched["bands"]
    flat16 = np.ascontiguousarray(
        np.asarray(hidden_states).reshape(B * S, H)).astype(np.float16)
    # W.T rows permuted so chunk `step` holds kb = 4*cg + step, cg=0..3.
    WT = np.asarray(W, dtype=np.float32).T.reshape(2 * HB, 128, D_OUT)
    wt_np = np.zeros((128, 4, 4, D_OUT), np.float32)
    for step in range(4):
        for cg in range(4):
            wt_np[:, step, cg, :] = WT[4 * cg + step]
    wt_np = np.ascontiguousarray(wt_np).astype(np.float16)
    brep_np = np.ascontiguousarray(
        np.broadcast_to(np.asarray(b, dtype=np.float32), (ncap, D_OUT)))
    fold_np = np.zeros((128, ncap), np.float16)
    for cg in range(4):
        for j in range(ncap):
            fold_np[32 * cg + j, j] = 1.0

    in_maps = []
    for c in range(NCORES):
        tok = flat16[sched["tok_idx"][c]]                      # [ntok, H]
        arr = np.ascontiguousarray(
            tok.T.reshape(HB, 128, ntok).transpose(1, 0, 2))   # [128, HB, ntok]
        pieces = []
        for k, (j0, j1, o0, lb, lv) in enumerate(bands):
            ns = j1 - j0
            ba = arr[:, :, o0:o0 + ns * lb]                    # [128, HB, ns*lb]
            # token-major: [128, lb, HB*ns]
            ba = ba.reshape(128, HB, ns, lb).transpose(0, 3, 1, 2)
            pieces.append(np.ascontiguousarray(ba).reshape(128, -1))
        hid_np = np.concatenate(pieces, axis=1)
        alpha_np = np.ascontiguousarray(np.broadcast_to(
            sched["alpha"][c][None, None, :], (128, HB, ncap)))
        beta_np = np.ascontiguousarray(np.broadcast_to(
            sched["beta"][c][None, :], (128, ncap)))
        in_maps.append({
            "hid": np.ascontiguousarray(hid_np),
            "wt": wt_np,
            "alpha": alpha_np,
            "beta": beta_np,
            "brep": brep_np,
            "fold": fold_np,
        })
    return in_maps


def kernel(hidden_states, W, b, turns, parts):
    parts = np.asarray(parts)
    turns = np.asarray(turns)

    sched = _build_schedule(parts, turns)
    nc = _build_program(sched)
    in_maps = _build_in_maps(sched, hidden_states, W, b)

    res = run_bass_kernel_spmd(nc, in_maps, list(range(NCORES)))

    full = np.zeros((sched["nrows"], D_OUT), dtype=np.float32)
    for c in range(NCORES):
        oc = res.results[c]["out"]
        for j in range(sched["ncap"]):
            g = sched["out_map"][c, j]
            if g >= 0:
                full[g] = oc[j]
    return full


# revision 16
# speedup vs baseline: 1.6301x; 1.2400x over previous
"""Trainium2 Bass kernel v4 (token-partition hybrid) for nn_BartPooler.

Layout: groups of G=8 consecutive tokens per partition row, [128, G*H] f16
tiles.  Per tile: VectorE computes a pair-sum (L1, fp16 2x mode) and a
3-level in-partition max tree; TensorE contracts the pair-sums against a
membership matrix (alpha and dup-token compensation folded into the
weights) for the segment means, and transposes the per-partition group
maxes; VectorE finishes the per-slot max over group columns.  Final GEMM:
16 k-blocks packed 4-up into PE column quadrants + fold, then bias+tanh.
"""

import numpy as np

import concourse.bacc as bacc
import concourse.mybir as mybir
import concourse.tile as tile
from concourse.bass_utils import run_bass_kernel_spmd
from concourse.masks import make_identity

NCORES = 8
B, S, H, T = 16, 4096, 1024, 16
D_OUT = 1024
HB = H // 128
G = 8            # tokens per partition row

F32 = mybir.dt.float32
F16 = mybir.dt.float16


def _groups_needed(cnt):
    g = -(-cnt // G)
    if cnt % G:
        g += 1       # ensure at least one pure-dup group for compensation
    return g


def _build_schedule(parts, turns):
    Bn, Tn = parts.shape
    segs = []
    for b in range(Bn):
        cum = 0
        for j in range(Tn):
            c = int(parts[b, j])
            if j < int(turns[b]):
                segs.append((b * Tn + j, b, 1 + cum, c))
            cum += c

    order = sorted(range(len(segs)), key=lambda i: -segs[i][3])
    core_slots = [[] for _ in range(NCORES)]
    for rank, i in enumerate(order):
        core_slots[rank % NCORES].append(segs[i])
    ncap = max(len(s) for s in core_slots)
    assert ncap <= 32, ncap

    NG = [max(_groups_needed(core_slots[c][j][3]) for c in range(NCORES)
              if j < len(core_slots[c])) for j in range(ncap)]
    AG = np.concatenate([[0], np.cumsum(NG)]).astype(np.int64)
    ngroups = int(AG[-1])
    ntiles = -(-ngroups // 128)
    ngpad = ntiles * 128
    ntok = ngpad * G

    tok_idx = np.zeros((NCORES, ntok), dtype=np.int64)
    mem = np.zeros((NCORES, 128, ntiles, ncap), dtype=np.float32)
    out_map = np.full((NCORES, ncap), -1, dtype=np.int64)
    for c in range(NCORES):
        for j, (grow, b, s0, cnt) in enumerate(core_slots[c]):
            out_map[c, j] = grow
            g0 = int(AG[j])
            lb = int(NG[j]) * G
            base = b * S + s0
            a = g0 * G
            tok_idx[c, a:a + cnt] = np.arange(base, base + cnt)
            tok_idx[c, a + cnt:a + lb] = base              # dup first token
            # membership weights: real/mixed groups 1/cnt; pure-dup groups
            # -r/(npure*G*cnt) so the r dups in the mixed group cancel.
            inv = 1.0 / cnt
            nfull, rem = divmod(cnt, G)
            nreal = nfull + (1 if rem else 0)
            npure = int(NG[j]) - nreal
            r = (G - rem) % G
            bw = -r / (npure * G) * inv if (npure and r) else 0.0
            for g in range(g0, g0 + nreal):
                mem[c, g % 128, g // 128, j] = inv
            for g in range(g0 + nreal, g0 + int(NG[j])):
                mem[c, g % 128, g // 128, j] = bw
    return {
        "ncap": ncap, "NG": NG, "AG": AG, "ngroups": ngroups,
        "ntiles": ntiles, "ntok": ntok,
        "tok_idx": tok_idx, "mem": mem,
        "out_map": out_map, "nrows": Bn * Tn,
    }


def _build_program(sched):
    ncap, ntiles = sched["ncap"], sched["ntiles"]
    AG, NG = sched["AG"], sched["NG"]

    nc = bacc.Bacc("TRN2", target_bir_lowering=False, debug=False,
                   num_devices=NCORES)
    hid = nc.dram_tensor("hid", [128, ntiles, G * H], F16,
                         kind="ExternalInput")
    mem = nc.dram_tensor("mem", [128, ntiles, ncap], F16,
                         kind="ExternalInput")
    wt = nc.dram_tensor("wt", [128, 4, 4, D_OUT], F16, kind="ExternalInput")
    brep = nc.dram_tensor("brep", [ncap, D_OUT], F32, kind="ExternalInput")
    fold = nc.dram_tensor("fold", [128, ncap], F16, kind="ExternalInput")
    out = nc.dram_tensor("out", [ncap, D_OUT], F32, kind="ExternalOutput")

    with tile.TileContext(nc) as tc:
        with (
            tc.tile_pool(name="main", bufs=1) as mp,
            tc.tile_pool(name="psum", bufs=1, space="PSUM") as pp,
            tc.tile_pool(name="trp", bufs=1, space="PSUM") as trpool,
        ):
            ident = mp.tile([128, 128], F16)
            make_identity(nc, ident[:])
            mem_sb = mp.tile([128, ntiles, ncap], F16)
            nc.scalar.dma_start(out=mem_sb[:], in_=mem[:])
            brep_sb = mp.tile([ncap, D_OUT], F32)
            nc.scalar.dma_start(out=brep_sb[:], in_=brep[:])
            fold_sb = mp.tile([128, ncap], F16)
            nc.scalar.dma_start(out=fold_sb[:], in_=fold[:])

            tiles = []
            for t in range(ntiles):
                ht = mp.tile([128, G * H], F16, name=f"ht{t}", tag=f"ht{t}")
                nc.sync.dma_start(out=ht[:], in_=hid[:, t, :])
                tiles.append(ht)
            wt_sb = mp.tile([128, 4, 4, D_OUT], F16)
            for step in range(4):
                nc.sync.dma_start(out=wt_sb[:, step], in_=wt[:, step])

            trmax = mp.tile([128, HB, ntiles * 128], F16)
            maxT = mp.tile([128, HB, ncap], F16)
            meansT = mp.tile([128, HB, ncap], F16)
            sum_ps = pp.tile([ncap, D_OUT], F32, name="sum_ps")

            cover = [[] for _ in range(ntiles)]
            for j in range(ncap):
                cover[(int(AG[j]) + int(NG[j]) - 1) // 128].append(j)

            for t, ht in enumerate(tiles):
                half = G // 2 * H
                ssc = mp.tile([128, half], F16, name=f"ssc{t}", tag=f"ssc{t}")
                # sum L1 (token i + token i+G/2), then PE membership matmuls
                nc.vector.tensor_tensor(out=ssc[:], in0=ht[:, :half],
                                        in1=ht[:, half:],
                                        op=mybir.AluOpType.add)
                for pos in range(G // 2):
                    for nhh in range(2):
                        nc.tensor.matmul(
                            sum_ps[:, nhh * 512:(nhh + 1) * 512],
                            lhsT=mem_sb[:, t, :],
                            rhs=ssc[:, pos * H + nhh * 512:
                                    pos * H + nhh * 512 + 512],
                            start=(t == 0 and pos == 0),
                            stop=(t == ntiles - 1 and pos == G // 2 - 1),
                        )
                # max tree in place on ht: G -> 1 per partition
                m = G * H
                for _ in range(3):
                    nc.vector.tensor_tensor(out=ht[:, :m // 2],
                                            in0=ht[:, :m // 2],
                                            in1=ht[:, m // 2:m],
                                            op=mybir.AluOpType.max)
                    m //= 2
                # transpose gmax [128, H] -> trmax group columns
                trp = trpool.tile([128, H], F16, tag="trp")
                for hb in range(HB):
                    nc.tensor.transpose(trp[:, hb * 128:(hb + 1) * 128],
                                        ht[:, hb * 128:(hb + 1) * 128],
                                        ident[:])
                nc.scalar.copy(
                    out=trmax[:, :, t * 128:(t + 1) * 128],
                    in_=trp[:].rearrange("p (b g) -> p b g", g=128))
                for j in cover[t]:
                    a, l = int(AG[j]), int(NG[j])
                    nc.vector.reduce_max(out=maxT[:, :, j:j + 1],
                                         in_=trmax[:, :, a:a + l],
                                         axis=mybir.AxisListType.X)

            # means: PSUM -> SBUF f16, transpose to [h, slot]
            means_s = mp.tile([ncap, D_OUT], F16)
            nc.scalar.copy(out=means_s[:], in_=sum_ps[:])
            tr2 = trpool.tile([128, HB * ncap], F16, tag="tr2")
            for hb in range(HB):
                nc.tensor.transpose(tr2[:, hb * ncap:(hb + 1) * ncap],
                                    means_s[:, hb * 128:(hb + 1) * 128],
                                    ident[:ncap, :ncap])
            nc.scalar.copy(out=meansT[:],
                           in_=tr2[:].rearrange("p (b j) -> p b j", j=ncap))

            # GEMM (4-up quadrant packing + fold), W chunk-pipelined
            osb = mp.tile([ncap, D_OUT], F32)
            for nh in range(2):
                nsl = slice(nh * 512, (nh + 1) * 512)
                gem_ps = pp.tile([128, 512], F32, name=f"gem{nh}")
                for step in range(4):
                    for cg in range(4):
                        kb = 4 * cg + step
                        lhsT = (maxT[:, kb, :] if kb < HB
                                else meansT[:, kb - HB, :])
                        nc.tensor.matmul(
                            gem_ps[32 * cg:32 * cg + ncap, :],
                            lhsT=lhsT,
                            rhs=wt_sb[:, step, cg, nsl],
                            start=(step == 0),
                            stop=(step == 3),
                            tile_position=(0, 32 * cg),
                        )
                gem_sb = mp.tile([128, 512], F16, name=f"gsb{nh}")
                nc.scalar.copy(out=gem_sb[:], in_=gem_ps[:])
                fold_ps = pp.tile([ncap, 512], F32, name=f"fps{nh}")
                nc.tensor.matmul(fold_ps[:], lhsT=fold_sb[:, :ncap],
                                 rhs=gem_sb[:], start=True, stop=True)
                nc.vector.tensor_add(out=osb[:, nsl], in0=fold_ps[:],
                                     in1=brep_sb[:, nsl])
            nc.scalar.activation(osb[:], osb[:],
                                 mybir.ActivationFunctionType.Tanh)
            nc.sync.dma_start(out=out[:], in_=osb[:])

    nc.compile()
    return nc


def _build_in_maps(sched, hidden_states, W, b):
    ncap, ntiles, ntok = sched["ncap"], sched["ntiles"], sched["ntok"]
    flat16 = np.ascontiguousarray(
        np.asarray(hidden_states).reshape(B * S, H)).astype(np.float16)
    WT = np.asarray(W, dtype=np.float32).T.reshape(2 * HB, 128, D_OUT)
    wt_np = np.zeros((128, 4, 4, D_OUT), np.float32)
    for step in range(4):
        for cg in range(4):
            wt_np[:, step, cg, :] = WT[4 * cg + step]
    wt_np = np.ascontiguousarray(wt_np).astype(np.float16)
    brep_np = np.ascontiguousarray(
        np.broadcast_to(np.asarray(b, dtype=np.float32), (ncap, D_OUT)))
    fold_np = np.zeros((128, ncap), np.float16)
    for cg in range(4):
        for j in range(ncap):
            fold_np[32 * cg + j, j] = 1.0

    in_maps = []
    for c in range(NCORES):
        tok = flat16[sched["tok_idx"][c]]                # [ntok, H]
        hid_np = np.ascontiguousarray(
            tok.reshape(ntiles, 128, G * H).transpose(1, 0, 2))
        in_maps.append({
            "hid": hid_np,
            "mem": np.ascontiguousarray(sched["mem"][c]).astype(np.float16),
            "wt": wt_np,
            "brep": brep_np,
            "fold": fold_np,
        })
    return in_maps


def kernel(hidden_states, W, b, turns, parts):
    parts = np.asarray(parts)
    turns = np.asarray(turns)

    sched = _build_schedule(parts, turns)
    nc = _build_program(sched)
    in_maps = _build_in_maps(sched, hidden_states, W, b)

    res = run_bass_kernel_spmd(nc, in_maps, list(range(NCORES)))

    full = np.zeros((sched["nrows"], D_OUT), dtype=np.float32)
    for c in range(NCORES):
        oc = res.results[c]["out"]
        for j in range(sched["ncap"]):
            g = sched["out_map"][c, j]
            if g >= 0:
                full[g] = oc[j]
    return full
